# revision 28
# baseline (speedup 1.0000x reference)
"""DotProductAttentionPooling on 8 trn2 NeuronCores.

reference:
    scores = einsum("bld,d->bl", x, q) / sqrt(D)
    scores = where(mask, scores, -inf)
    attn   = nan_to_num(softmax(scores, axis=-1))
    out    = einsum("bl,bld->bd", attn, x)            # [B, D]

Strategy (memory-bound: x is 256 MiB and must be read exactly once):
  - Data-parallel: batch B=32 sharded 4-per-core across 8 cores; query
    replicated; output [B, D] gathered on host.
  - x[b] streams to SBUF with l = p*64 + i so each partition's HBM read
    is one contiguous run. The DMA itself casts fp32 -> fp16 (SWDGE
    inline convert) so no engine spends cycles on conversion and the
    DMA runs gapless at the HBM roofline.
  - Scores, split across engines per quarter: one dense fp16
    tensor_tensor product against the pre-scaled query read through a
    stride-0 broadcast AP (2x DVE mode, no replication needed);
    chunks [0, ND) reduced by a DVE pairwise-add tree
    + small 1x reduce, chunks [ND, QC) by ScalarE ACTIVATE-with-accum
    (free-dim sum) so DVE and ScalarE both sit under the ~5us/quarter
    DMA pace. (tensor_tensor_reduce would fuse product+reduce but its
    ucode faults the exec unit on this HW; tensor_scalar+accum_out is
    rejected by the walrus verifier; scalar_tensor_tensor+accum_out
    works but only at 1x rate, which is slower than the tree.)
  - Softmax without max-subtraction: scores are O(0.1) so exp cannot
    overflow; the -inf mask becomes a -40 additive bias (exp -> 0 in
    fp16). The 1/sqrt(D) scale is folded into the fp16 query.
  - Pooling: unnormalized acc[1, 256] += w_col.T @ x_chunk as fp16
    accumulating PE matmuls (contraction over partition dim = L);
    denominator via one extra tiny matmul per quarter (ones^T @ wqh
    into a [1, QC] PSUM tile). Final normalize on ScalarE out of PSUM,
    deferred one batch so it never stalls the pipeline.
  - Software pipelining: each quarter's tail (mask add, exp, matmuls)
    is emitted AFTER the next quarter's head (product + reduces), so
    no engine's in-order stream waits on a cross-engine result that
    isn't already done. All four batches' masks load in ONE permuted-AP
    DMA before the x stream saturates the SDMA engines (per-batch mask
    DMAs complete 8+us late behind the saturated stream and head-of-
    line-block DVE), and one CAST + tensor_scalar prepares the bias for
    the whole kernel. Steady state is DMA-paced: the x stream runs
    gapless at the HBM roofline (~81us for 32 MiB/core read) and every
    engine tracks it with ~one chain-latency of lag.
"""

import numpy as np

B, L, D = 32, 8192, 256
N_CORES = 8
BPC = B // N_CORES        # batches per core
P = 128                   # partitions
CHUNKS = L // P           # 64 L-chunks per batch
QC = 16                   # chunks per quarter tile
NQ = CHUNKS // QC         # quarters per batch
NQT = BPC * NQ            # total quarters per core
LOOKAHEAD = 12            # quarters of fp16 DMA prefetch
N_SC = 4                  # chunks per quarter reduced on ScalarE accum
ND = QC - N_SC            # chunks per quarter reduced on DVE tree
SCALE = 1.0 / float(np.sqrt(D))

_cache = {}


def _build():
    import concourse.bacc as bacc
    import concourse.bass as bass
    import concourse.tile as tile
    from concourse import mybir

    f32 = mybir.dt.float32
    f16 = mybir.dt.float16
    i32 = mybir.dt.int32
    nc = bacc.Bacc("TRN2", target_bir_lowering=False, debug=False,
                   num_devices=N_CORES)

    x = nc.declare_dram_parameter("x", [BPC, L, D], f32, isOutput=False)
    mask = nc.declare_dram_parameter("mask", [BPC, L], i32, isOutput=False)
    query = nc.declare_dram_parameter("query", [D], f32, isOutput=False)
    out = nc.declare_dram_parameter("out", [BPC, D], f32, isOutput=True)

    # l = p * CHUNKS + i: per-partition HBM reads are contiguous
    x_r = x[:].rearrange("b (p i) d -> b p i d", p=P)
    mask_r = mask[:].rearrange("b (p i) -> b p i", p=P)

    with tile.TileContext(nc) as tc:
        with (
            tc.tile_pool(name="xh", bufs=LOOKAHEAD + 2) as xhp,
            tc.tile_pool(name="prod", bufs=3) as prodp,
            tc.tile_pool(name="tree", bufs=4) as treep,
            tc.tile_pool(name="small", bufs=6) as small,
            tc.tile_pool(name="singles", bufs=1) as singles,
            tc.tile_pool(name="psum", bufs=2, space="PSUM") as psums,
        ):
            xh_tiles = {}       # quarter index -> staged fp16 tile
            state = {}          # per-batch softmax state
            pending = []        # deferred (pool_ps, den_ps, b)

            def issue_quarter(k):
                b, qi = divmod(k, NQ)
                if k == NQT - 1:
                    # final quarter: two separate half-tiles so the drain
                    # chain starts when the first half lands, ~2.6us
                    # before the last byte (deps are tile-granular)
                    halves = []
                    for h in range(2):
                        xh = xhp.tile([P, QC // 2, D], f16,
                                      tag="xh_half", bufs=2)
                        sl = slice(qi * QC + h * (QC // 2),
                                   qi * QC + (h + 1) * (QC // 2))
                        nc.gpsimd.dma_start(out=xh[:],
                                            in_=x_r[b, :, sl, :])
                        halves.append(xh)
                    xh_tiles[k] = halves
                    return
                xh = xhp.tile([P, QC, D], f16, tag="xh")
                # one DMA per quarter: splitting adds serialized SWDGE
                # descriptor-gen (~0.65us each) and the tile dep waits
                # on ALL sub-DMA sems, so the split DELAYS first compute
                sl = slice(qi * QC, (qi + 1) * QC)
                nc.gpsimd.dma_start(out=xh[:], in_=x_r[b, :, sl, :])
                xh_tiles[k] = xh

            def flush_one(entry):
                pool_ps, den_ps, bb = entry
                den_sum = small.tile([1, 1], f32, tag="den_sum",
                                     name=f"den_sum{bb}")
                nc.vector.tensor_reduce(out=den_sum[:], in_=den_ps[:],
                                        op=mybir.AluOpType.add,
                                        axis=mybir.AxisListType.X)
                # +1e-30 so an all-masked batch divides to 0, not NaN
                den_eps = small.tile([1, 1], f32, tag="den_eps",
                                     name=f"den_eps{bb}")
                nc.vector.tensor_scalar_add(den_eps[:], den_sum[:], 1e-30)
                rden = small.tile([1, 1], f32, tag="rden",
                                  name=f"rden{bb}")
                nc.vector.reciprocal(rden[:], den_eps[:])
                out_sb = small.tile([1, D], f32)
                nc.scalar.activation(
                    out=out_sb[:], in_=pool_ps[0:1, 0:D],
                    func=mybir.ActivationFunctionType.Copy,
                    scale=rden[0:1, 0:1])
                nc.sync.dma_start(out=out[bb:bb + 1, :], in_=out_sb[:])

            qstate = {}

            def emit_tail(j):
                bb, qj = divmod(j, NQ)
                stj = state[bb]
                xhj = xh_tiles.pop(j)
                scores_j = qstate.pop(j)
                sl = slice(bb * CHUNKS + qj * QC,
                           bb * CHUNKS + (qj + 1) * QC)
                scores_m = small.tile([P, QC], f32, tag="scores_m")
                nc.vector.tensor_tensor(out=scores_m[:], in0=scores_j[:],
                                        in1=mask_f_all[:, sl],
                                        op=mybir.AluOpType.add)
                wqh = small.tile([P, QC], f16, tag="wqh")
                nc.scalar.activation(out=wqh[:], in_=scores_m[:],
                                     func=mybir.ActivationFunctionType.Exp)
                # denominator: per-chunk weight sums accumulate in PSUM
                nc.tensor.matmul(stj["den_ps"][:], ones_col[:], wqh[:],
                                 start=(qj == 0), stop=(qj == NQ - 1))
                for i in range(QC):
                    nc.tensor.matmul(
                        stj["pool_ps"][:],
                        wqh[:, i:i + 1],
                        xhj[:, i, :],
                        start=(qj == 0 and i == 0),
                        stop=(qj == NQ - 1 and i == QC - 1),
                    )
                if qj == NQ - 1:
                    pending.append((stj["pool_ps"], stj["den_ps"], bb))
                    del state[bb]
                    # flush all but the newest: batch bb's epilogue runs
                    # ~a batch after its last matmul, never stalling
                    while len(pending) > 1:
                        flush_one(pending.pop(0))

            # broadcast query across partitions with a rank-1 PE matmul
            # (ones[1,128]^T @ q[1,256]) — a SWDGE broadcast DMA would cost
            # ~15us of head-of-line latency before the first compute
            q_row = singles.tile([1, D], f32)
            nc.sync.dma_start(out=q_row[:],
                              in_=query[:].rearrange("(o d) -> o d", o=1))
            ones_row = singles.tile([1, P], f32)
            nc.vector.memset(ones_row[:], 1.0)
            ones_col = singles.tile([P, 1], f16)
            nc.vector.memset(ones_col[:], 1.0)
            q_ps = psums.tile([P, D], f32, tag="qbc", name="q_ps")
            nc.tensor.matmul(q_ps[:], ones_row[:], q_row[:],
                             start=True, stop=True)
            # pre-scaled fp16 query (softmax 1/sqrt(D) folded in)
            qs = singles.tile([P, D], f16)
            nc.scalar.activation(out=qs[:], in_=q_ps[:],
                                 func=mybir.ActivationFunctionType.Copy,
                                 scale=SCALE)
            # all four batches' masks land before the x stream saturates
            # the SDMA engines; one CAST + one tensor_scalar prepares the
            # additive bias (0 kept, -40 masked) for the whole kernel
            mask_i_all = singles.tile([P, BPC * CHUNKS], i32)
            nc.sync.dma_start(
                out=mask_i_all[:],
                in_=mask[:].rearrange("b (p i) -> p b i", p=P))
            mask_f_all = singles.tile([P, BPC * CHUNKS], f32)
            nc.vector.tensor_copy(mask_f_all[:], mask_i_all[:])
            nc.vector.tensor_scalar(
                out=mask_f_all[:], in0=mask_f_all[:],
                scalar1=40.0, scalar2=40.0,
                op0=mybir.AluOpType.mult,
                op1=mybir.AluOpType.subtract)



            for k in range(min(LOOKAHEAD, NQT)):
                issue_quarter(k)

            for k in range(NQT):
                b, qi = divmod(k, NQ)
                if k == NQT - 1:
                    # drain quarter k-1 (exp + matmuls) BEFORE the last
                    # head: otherwise ScalarE's in-order stream parks
                    # exp(k-1) behind accums(k), which wait on the final
                    # quarter's DMA — stalling TensorE's drain by ~3us
                    emit_tail(k - 1)
                    # final quarter, processed per half-tile: score,
                    # exp and pool each half as soon as it lands
                    stf = state[b]
                    halves = xh_tiles.pop(k)
                    HC = QC // 2
                    NDH, NSH = 6, 2
                    wqh_full = small.tile([P, QC], f16, tag="wqh")
                    for h in range(2):
                        xhh = halves[h]
                        sco = small.tile([P, HC], f32, tag="scores",
                                         name=f"scores_h{h}")
                        prodh = prodp.tile([P, HC, D], f16,
                                           tag="prodh_h", bufs=2)
                        nc.vector.tensor_tensor(
                            out=prodh[:], in0=xhh[:],
                            in1=qs[:].rearrange("p (o d) -> p o d", o=1)
                            .to_broadcast([P, HC, D]),
                            op=mybir.AluOpType.mult)
                        t1 = treep.tile([P, NDH, 128], f16, tag="t1h",
                                        bufs=2)
                        nc.vector.tensor_tensor(
                            out=t1[:], in0=prodh[:, 0:NDH, 0:128],
                            in1=prodh[:, 0:NDH, 128:256],
                            op=mybir.AluOpType.add)
                        t2 = treep.tile([P, NDH, 64], f16, tag="t2h",
                                        bufs=2)
                        nc.vector.tensor_tensor(
                            out=t2[:], in0=t1[:, :, 0:64],
                            in1=t1[:, :, 64:128],
                            op=mybir.AluOpType.add)
                        t3 = treep.tile([P, NDH, 32], f16, tag="t3h",
                                        bufs=2)
                        nc.vector.tensor_tensor(
                            out=t3[:], in0=t2[:, :, 0:32],
                            in1=t2[:, :, 32:64],
                            op=mybir.AluOpType.add)
                        nc.vector.tensor_reduce(
                            out=sco[:, 0:NDH], in_=t3[:],
                            op=mybir.AluOpType.add,
                            axis=mybir.AxisListType.X)
                        sch = treep.tile([P, NSH, D], f16, tag="sch",
                                         bufs=2)
                        for i in range(NSH):
                            nc.scalar.activation(
                                out=sch[:, i, :],
                                in_=prodh[:, NDH + i, :],
                                func=mybir.ActivationFunctionType.Copy,
                                accum_out=sco[:, NDH + i:NDH + i + 1])
                        slm = slice(b * CHUNKS + qi * QC + h * HC,
                                    b * CHUNKS + qi * QC + (h + 1) * HC)
                        smh = small.tile([P, HC], f32, tag="scores_m",
                                         name=f"sm_h{h}")
                        nc.vector.tensor_tensor(
                            out=smh[:], in0=sco[:],
                            in1=mask_f_all[:, slm],
                            op=mybir.AluOpType.add)
                        nc.scalar.activation(
                            out=wqh_full[:, h * HC:(h + 1) * HC],
                            in_=smh[:],
                            func=mybir.ActivationFunctionType.Exp)
                        for i in range(HC):
                            nc.tensor.matmul(
                                stf["pool_ps"][:],
                                wqh_full[:, h * HC + i:h * HC + i + 1],
                                xhh[:, i, :],
                                start=False,
                                stop=(h == 1 and i == HC - 1))
                    nc.tensor.matmul(stf["den_ps"][:], ones_col[:],
                                     wqh_full[:], start=False, stop=True)
                    pending.append((stf["pool_ps"], stf["den_ps"], b))
                    del state[b]
                    break
                if qi == 0:
                    state[b] = {
                        "pool_ps": psums.tile([1, D], f32, tag="pool",
                                              name=f"pool_ps{b}", bufs=3),
                        "den_ps": psums.tile([1, QC], f32, tag="den",
                                             name=f"den_ps{b}", bufs=3),
                    }
                # head: product + per-chunk reduces (DVE tree for
                # chunks [0, ND), ScalarE ACT-accum for [ND, QC))
                xh = xh_tiles[k]
                scores_q = small.tile([P, QC], f32, tag="scores",
                                      name=f"scores{k}")
                prodh = prodp.tile([P, QC, D], f16, tag="prodh")
                nc.vector.tensor_tensor(
                    out=prodh[:], in0=xh[:],
                    in1=qs[:].rearrange("p (o d) -> p o d", o=1)
                        .to_broadcast([P, QC, D]),
                    op=mybir.AluOpType.mult)
                t1 = treep.tile([P, ND, 128], f16, tag="t1")
                nc.vector.tensor_tensor(out=t1[:],
                                        in0=prodh[:, 0:ND, 0:128],
                                        in1=prodh[:, 0:ND, 128:256],
                                        op=mybir.AluOpType.add)
                t2 = treep.tile([P, ND, 64], f16, tag="t2")
                nc.vector.tensor_tensor(out=t2[:], in0=t1[:, :, 0:64],
                                        in1=t1[:, :, 64:128],
                                        op=mybir.AluOpType.add)
                t3 = treep.tile([P, ND, 32], f16, tag="t3")
                nc.vector.tensor_tensor(out=t3[:], in0=t2[:, :, 0:32],
                                        in1=t2[:, :, 32:64],
                                        op=mybir.AluOpType.add)
                nc.vector.tensor_reduce(out=scores_q[:, 0:ND], in_=t3[:],
                                        op=mybir.AluOpType.add,
                                        axis=mybir.AxisListType.X)
                sc_scr = treep.tile([P, N_SC, D], f16, tag="sc_scr")
                for i in range(N_SC):
                    nc.scalar.activation(
                        out=sc_scr[:, i, :], in_=prodh[:, ND + i, :],
                        func=mybir.ActivationFunctionType.Copy,
                        accum_out=scores_q[:, ND + i:ND + i + 1])
                qstate[k] = scores_q

                if k + LOOKAHEAD < NQT:
                    issue_quarter(k + LOOKAHEAD)
                if 0 < k < NQT - 1:
                    emit_tail(k - 1)
            while pending:
                flush_one(pending.pop(0))

    nc.compile()
    return nc


def kernel(x: np.ndarray, mask: np.ndarray, query: np.ndarray) -> np.ndarray:
    from concourse.bass_utils import run_bass_kernel_spmd

    if "nc" not in _cache:
        _cache["nc"] = _build()
    nc = _cache["nc"]

    x = np.ascontiguousarray(np.asarray(x, dtype=np.float32))
    mask = np.ascontiguousarray(np.asarray(mask, dtype=np.int32))
    query = np.ascontiguousarray(np.asarray(query, dtype=np.float32))

    in_maps = [
        {
            "x": np.ascontiguousarray(x[c * BPC:(c + 1) * BPC]),
            "mask": np.ascontiguousarray(mask[c * BPC:(c + 1) * BPC]),
            "query": query,
        }
        for c in range(N_CORES)
    ]
    res = run_bass_kernel_spmd(nc, in_maps, core_ids=list(range(N_CORES)))
    return np.concatenate([res.results[c]["out"] for c in range(N_CORES)], axis=0)


# revision 29
# speedup vs baseline: 1.0775x; 1.0775x over previous
"""DotProductAttentionPooling on 8 trn2 NeuronCores.

reference:
    scores = einsum("bld,d->bl", x, q) / sqrt(D)
    scores = where(mask, scores, -inf)
    attn   = nan_to_num(softmax(scores, axis=-1))
    out    = einsum("bl,bld->bd", attn, x)            # [B, D]

Strategy (memory-bound: x is 256 MiB and must be read exactly once):
  - Data-parallel: batch B=32 sharded 4-per-core across 8 cores; query
    replicated; output [B, D] gathered on host.
  - x[b] streams to SBUF with l = p*64 + i so each partition's HBM read
    is one contiguous run. The DMA itself casts fp32 -> fp16 (SWDGE
    inline convert) so no engine spends cycles on conversion and the
    DMA runs gapless at the HBM roofline.
  - Scores, split across engines per quarter: one dense fp16
    tensor_tensor product against the pre-scaled query read through a
    stride-0 broadcast AP (2x DVE mode, no replication needed);
    chunks [0, ND) reduced by a DVE pairwise-add tree
    + small 1x reduce, chunks [ND, QC) by ScalarE ACTIVATE-with-accum
    (free-dim sum) so DVE and ScalarE both sit under the ~5us/quarter
    DMA pace. (tensor_tensor_reduce would fuse product+reduce but its
    ucode faults the exec unit on this HW; tensor_scalar+accum_out is
    rejected by the walrus verifier; scalar_tensor_tensor+accum_out
    works but only at 1x rate, which is slower than the tree.)
  - Softmax without max-subtraction: scores are O(0.1) so exp cannot
    overflow; the -inf mask becomes a -40 additive bias (exp -> 0 in
    fp16). The 1/sqrt(D) scale is folded into the fp16 query.
  - Pooling: unnormalized acc[1, 256] += w_col.T @ x_chunk as fp16
    accumulating PE matmuls (contraction over partition dim = L);
    denominator via one extra tiny matmul per quarter (ones^T @ wqh
    into a [1, QC] PSUM tile). Final normalize on ScalarE out of PSUM,
    deferred one batch so it never stalls the pipeline.
  - Software pipelining: each quarter's tail (mask add, exp, matmuls)
    is emitted AFTER the next quarter's head (product + reduces), so
    no engine's in-order stream waits on a cross-engine result that
    isn't already done. All four batches' masks load in ONE permuted-AP
    DMA before the x stream saturates the SDMA engines (per-batch mask
    DMAs complete 8+us late behind the saturated stream and head-of-
    line-block DVE), and one CAST + tensor_scalar prepares the bias for
    the whole kernel. Steady state is DMA-paced: the x stream runs
    gapless at the HBM roofline (~81us for 32 MiB/core read) and every
    engine tracks it with ~one chain-latency of lag.
"""

import numpy as np

B, L, D = 32, 8192, 256
N_CORES = 8
BPC = B // N_CORES        # batches per core
P = 128                   # partitions
CHUNKS = L // P           # 64 L-chunks per batch
QC = 16                   # chunks per quarter tile
NQ = CHUNKS // QC         # quarters per batch
NQT = BPC * NQ            # total quarters per core
LOOKAHEAD = 12            # quarters of fp16 DMA prefetch
N_SC = 4                  # chunks per quarter reduced on ScalarE accum
ND = QC - N_SC            # chunks per quarter reduced on DVE tree
SCALE = 1.0 / float(np.sqrt(D))

_cache = {}


def _build():
    import concourse.bacc as bacc
    import concourse.bass as bass
    import concourse.tile as tile
    from concourse import mybir

    f32 = mybir.dt.float32
    f16 = mybir.dt.float16
    i32 = mybir.dt.int32
    nc = bacc.Bacc("TRN2", target_bir_lowering=False, debug=False,
                   num_devices=N_CORES)

    x = nc.declare_dram_parameter("x", [BPC, L, D], f32, isOutput=False)
    mask = nc.declare_dram_parameter("mask", [BPC, L], i32, isOutput=False)
    query = nc.declare_dram_parameter("query", [D], f32, isOutput=False)
    out = nc.declare_dram_parameter("out", [BPC, D], f32, isOutput=True)

    # l = p * CHUNKS + i: per-partition HBM reads are contiguous
    x_r = x[:].rearrange("b (p i) d -> b p i d", p=P)
    mask_r = mask[:].rearrange("b (p i) -> b p i", p=P)

    with tile.TileContext(nc) as tc:
        with (
            tc.tile_pool(name="xh", bufs=LOOKAHEAD + 2) as xhp,
            tc.tile_pool(name="prod", bufs=3) as prodp,
            tc.tile_pool(name="tree", bufs=4) as treep,
            tc.tile_pool(name="small", bufs=6) as small,
            tc.tile_pool(name="singles", bufs=1) as singles,
            tc.tile_pool(name="psum", bufs=2, space="PSUM") as psums,
        ):
            xh_tiles = {}       # quarter index -> staged fp16 tile
            state = {}          # per-batch softmax state
            pending = []        # deferred (pool_ps, den_ps, b)

            def issue_quarter(k):
                b, qi = divmod(k, NQ)
                if k == NQT - 1:
                    # final quarter: two separate half-tiles so the drain
                    # chain starts when the first half lands, ~2.6us
                    # before the last byte (deps are tile-granular)
                    halves = []
                    for h in range(2):
                        xh = xhp.tile([P, QC // 2, D], f16,
                                      tag="xh_half", bufs=2)
                        sl = slice(qi * QC + h * (QC // 2),
                                   qi * QC + (h + 1) * (QC // 2))
                        nc.gpsimd.dma_start(out=xh[:],
                                            in_=x_r[b, :, sl, :])
                        halves.append(xh)
                    xh_tiles[k] = halves
                    return
                xh = xhp.tile([P, QC, D], f16, tag="xh")
                # first quarters: finer DMA slices so the pipeline fills
                # fast; later ones: a single cheap descriptor push
                ndma = 4 if k == 0 else (2 if k == 1 else 1)
                step = QC // ndma
                for g in range(ndma):
                    sl = slice(qi * QC + g * step, qi * QC + (g + 1) * step)
                    nc.gpsimd.dma_start(
                        out=xh[:, g * step:(g + 1) * step, :],
                        in_=x_r[b, :, sl, :])
                xh_tiles[k] = xh

            def flush_one(entry):
                pool_ps, den_ps, bb = entry
                den_sum = small.tile([1, 1], f32, tag="den_sum",
                                     name=f"den_sum{bb}")
                nc.vector.tensor_reduce(out=den_sum[:], in_=den_ps[:],
                                        op=mybir.AluOpType.add,
                                        axis=mybir.AxisListType.X)
                # +1e-30 so an all-masked batch divides to 0, not NaN
                den_eps = small.tile([1, 1], f32, tag="den_eps",
                                     name=f"den_eps{bb}")
                nc.vector.tensor_scalar_add(den_eps[:], den_sum[:], 1e-30)
                rden = small.tile([1, 1], f32, tag="rden",
                                  name=f"rden{bb}")
                nc.vector.reciprocal(rden[:], den_eps[:])
                out_sb = small.tile([1, D], f32)
                nc.scalar.activation(
                    out=out_sb[:], in_=pool_ps[0:1, 0:D],
                    func=mybir.ActivationFunctionType.Copy,
                    scale=rden[0:1, 0:1])
                nc.sync.dma_start(out=out[bb:bb + 1, :], in_=out_sb[:])

            qstate = {}

            def emit_tail(j):
                bb, qj = divmod(j, NQ)
                stj = state[bb]
                xhj = xh_tiles.pop(j)
                scores_j = qstate.pop(j)
                sl = slice(bb * CHUNKS + qj * QC,
                           bb * CHUNKS + (qj + 1) * QC)
                scores_m = small.tile([P, QC], f32, tag="scores_m")
                nc.vector.tensor_tensor(out=scores_m[:], in0=scores_j[:],
                                        in1=mask_f_all[:, sl],
                                        op=mybir.AluOpType.add)
                wqh = small.tile([P, QC], f16, tag="wqh")
                nc.scalar.activation(out=wqh[:], in_=scores_m[:],
                                     func=mybir.ActivationFunctionType.Exp)
                # denominator: per-chunk weight sums accumulate in PSUM
                nc.tensor.matmul(stj["den_ps"][:], ones_col[:], wqh[:],
                                 start=(qj == 0), stop=(qj == NQ - 1))
                for i in range(QC):
                    nc.tensor.matmul(
                        stj["pool_ps"][:],
                        wqh[:, i:i + 1],
                        xhj[:, i, :],
                        start=(qj == 0 and i == 0),
                        stop=(qj == NQ - 1 and i == QC - 1),
                    )
                if qj == NQ - 1:
                    pending.append((stj["pool_ps"], stj["den_ps"], bb))
                    del state[bb]
                    # flush all but the newest: batch bb's epilogue runs
                    # ~a batch after its last matmul, never stalling
                    while len(pending) > 1:
                        flush_one(pending.pop(0))

            # broadcast query across partitions with a rank-1 PE matmul
            # (ones[1,128]^T @ q[1,256]) — a SWDGE broadcast DMA would cost
            # ~15us of head-of-line latency before the first compute
            q_row = singles.tile([1, D], f32)
            nc.sync.dma_start(out=q_row[:],
                              in_=query[:].rearrange("(o d) -> o d", o=1))
            ones_row = singles.tile([1, P], f32)
            nc.vector.memset(ones_row[:], 1.0)
            ones_col = singles.tile([P, 1], f16)
            nc.vector.memset(ones_col[:], 1.0)
            q_ps = psums.tile([P, D], f32, tag="qbc", name="q_ps")
            nc.tensor.matmul(q_ps[:], ones_row[:], q_row[:],
                             start=True, stop=True)
            # pre-scaled fp16 query (softmax 1/sqrt(D) folded in)
            qs = singles.tile([P, D], f16)
            nc.scalar.activation(out=qs[:], in_=q_ps[:],
                                 func=mybir.ActivationFunctionType.Copy,
                                 scale=SCALE)
            # all four batches' masks land before the x stream saturates
            # the SDMA engines; one CAST + one tensor_scalar prepares the
            # additive bias (0 kept, -40 masked) for the whole kernel
            mask_i_all = singles.tile([P, BPC * CHUNKS], i32)
            nc.sync.dma_start(
                out=mask_i_all[:],
                in_=mask[:].rearrange("b (p i) -> p b i", p=P))
            mask_f_all = singles.tile([P, BPC * CHUNKS], f32)
            nc.vector.tensor_copy(mask_f_all[:], mask_i_all[:])
            nc.vector.tensor_scalar(
                out=mask_f_all[:], in0=mask_f_all[:],
                scalar1=40.0, scalar2=40.0,
                op0=mybir.AluOpType.mult,
                op1=mybir.AluOpType.subtract)



            for k in range(min(LOOKAHEAD, NQT)):
                issue_quarter(k)

            for k in range(NQT):
                b, qi = divmod(k, NQ)
                if k == NQT - 1:
                    # drain quarter k-1 (exp + matmuls) BEFORE the last
                    # head: otherwise ScalarE's in-order stream parks
                    # exp(k-1) behind accums(k), which wait on the final
                    # quarter's DMA — stalling TensorE's drain by ~3us
                    emit_tail(k - 1)
                    # final quarter, processed per half-tile: score,
                    # exp and pool each half as soon as it lands
                    stf = state[b]
                    halves = xh_tiles.pop(k)
                    HC = QC // 2
                    NDH, NSH = 6, 2
                    wqh_full = small.tile([P, QC], f16, tag="wqh")
                    for h in range(2):
                        xhh = halves[h]
                        sco = small.tile([P, HC], f32, tag="scores",
                                         name=f"scores_h{h}")
                        prodh = prodp.tile([P, HC, D], f16,
                                           tag="prodh_h", bufs=2)
                        nc.vector.tensor_tensor(
                            out=prodh[:], in0=xhh[:],
                            in1=qs[:].rearrange("p (o d) -> p o d", o=1)
                            .to_broadcast([P, HC, D]),
                            op=mybir.AluOpType.mult)
                        t1 = treep.tile([P, NDH, 128], f16, tag="t1h",
                                        bufs=2)
                        nc.vector.tensor_tensor(
                            out=t1[:], in0=prodh[:, 0:NDH, 0:128],
                            in1=prodh[:, 0:NDH, 128:256],
                            op=mybir.AluOpType.add)
                        t2 = treep.tile([P, NDH, 64], f16, tag="t2h",
                                        bufs=2)
                        nc.vector.tensor_tensor(
                            out=t2[:], in0=t1[:, :, 0:64],
                            in1=t1[:, :, 64:128],
                            op=mybir.AluOpType.add)
                        t3 = treep.tile([P, NDH, 32], f16, tag="t3h",
                                        bufs=2)
                        nc.vector.tensor_tensor(
                            out=t3[:], in0=t2[:, :, 0:32],
                            in1=t2[:, :, 32:64],
                            op=mybir.AluOpType.add)
                        nc.vector.tensor_reduce(
                            out=sco[:, 0:NDH], in_=t3[:],
                            op=mybir.AluOpType.add,
                            axis=mybir.AxisListType.X)
                        sch = treep.tile([P, NSH, D], f16, tag="sch",
                                         bufs=2)
                        for i in range(NSH):
                            nc.scalar.activation(
                                out=sch[:, i, :],
                                in_=prodh[:, NDH + i, :],
                                func=mybir.ActivationFunctionType.Copy,
                                accum_out=sco[:, NDH + i:NDH + i + 1])
                        slm = slice(b * CHUNKS + qi * QC + h * HC,
                                    b * CHUNKS + qi * QC + (h + 1) * HC)
                        smh = small.tile([P, HC], f32, tag="scores_m",
                                         name=f"sm_h{h}")
                        nc.vector.tensor_tensor(
                            out=smh[:], in0=sco[:],
                            in1=mask_f_all[:, slm],
                            op=mybir.AluOpType.add)
                        nc.scalar.activation(
                            out=wqh_full[:, h * HC:(h + 1) * HC],
                            in_=smh[:],
                            func=mybir.ActivationFunctionType.Exp)
                        for i in range(HC):
                            nc.tensor.matmul(
                                stf["pool_ps"][:],
                                wqh_full[:, h * HC + i:h * HC + i + 1],
                                xhh[:, i, :],
                                start=False,
                                stop=(h == 1 and i == HC - 1))
                    nc.tensor.matmul(stf["den_ps"][:], ones_col[:],
                                     wqh_full[:], start=False, stop=True)
                    pending.append((stf["pool_ps"], stf["den_ps"], b))
                    del state[b]
                    break
                if qi == 0:
                    state[b] = {
                        "pool_ps": psums.tile([1, D], f32, tag="pool",
                                              name=f"pool_ps{b}", bufs=3),
                        "den_ps": psums.tile([1, QC], f32, tag="den",
                                             name=f"den_ps{b}", bufs=3),
                    }
                # head: product + per-chunk reduces (DVE tree for
                # chunks [0, ND), ScalarE ACT-accum for [ND, QC))
                xh = xh_tiles[k]
                scores_q = small.tile([P, QC], f32, tag="scores",
                                      name=f"scores{k}")
                prodh = prodp.tile([P, QC, D], f16, tag="prodh")
                nc.vector.tensor_tensor(
                    out=prodh[:], in0=xh[:],
                    in1=qs[:].rearrange("p (o d) -> p o d", o=1)
                        .to_broadcast([P, QC, D]),
                    op=mybir.AluOpType.mult)
                t1 = treep.tile([P, ND, 128], f16, tag="t1")
                nc.vector.tensor_tensor(out=t1[:],
                                        in0=prodh[:, 0:ND, 0:128],
                                        in1=prodh[:, 0:ND, 128:256],
                                        op=mybir.AluOpType.add)
                t2 = treep.tile([P, ND, 64], f16, tag="t2")
                nc.vector.tensor_tensor(out=t2[:], in0=t1[:, :, 0:64],
                                        in1=t1[:, :, 64:128],
                                        op=mybir.AluOpType.add)
                t3 = treep.tile([P, ND, 32], f16, tag="t3")
                nc.vector.tensor_tensor(out=t3[:], in0=t2[:, :, 0:32],
                                        in1=t2[:, :, 32:64],
                                        op=mybir.AluOpType.add)
                nc.vector.tensor_reduce(out=scores_q[:, 0:ND], in_=t3[:],
                                        op=mybir.AluOpType.add,
                                        axis=mybir.AxisListType.X)
                sc_scr = treep.tile([P, N_SC, D], f16, tag="sc_scr")
                for i in range(N_SC):
                    nc.scalar.activation(
                        out=sc_scr[:, i, :], in_=prodh[:, ND + i, :],
                        func=mybir.ActivationFunctionType.Copy,
                        accum_out=scores_q[:, ND + i:ND + i + 1])
                qstate[k] = scores_q

                if k + LOOKAHEAD < NQT:
                    issue_quarter(k + LOOKAHEAD)
                if 0 < k < NQT - 1:
                    emit_tail(k - 1)
            while pending:
                flush_one(pending.pop(0))

    nc.compile()
    return nc


def kernel(x: np.ndarray, mask: np.ndarray, query: np.ndarray) -> np.ndarray:
    from concourse.bass_utils import run_bass_kernel_spmd

    if "nc" not in _cache:
        _cache["nc"] = _build()
    nc = _cache["nc"]

    x = np.ascontiguousarray(np.asarray(x, dtype=np.float32))
    mask = np.ascontiguousarray(np.asarray(mask, dtype=np.int32))
    query = np.ascontiguousarray(np.asarray(query, dtype=np.float32))

    in_maps = [
        {
            "x": np.ascontiguousarray(x[c * BPC:(c + 1) * BPC]),
            "mask": np.ascontiguousarray(mask[c * BPC:(c + 1) * BPC]),
            "query": query,
        }
        for c in range(N_CORES)
    ]
    res = run_bass_kernel_spmd(nc, in_maps, core_ids=list(range(N_CORES)))
    return np.concatenate([res.results[c]["out"] for c in range(N_CORES)], axis=0)
